# revision 14
# baseline (speedup 1.0000x reference)
"""Trainium2 Bass kernel for nn_GPU_Actor (gnn_message_passing).

Math (H=1 collapses the whole network to per-row scalars):
  Edot[b,i] = expert_node[b,i,:] . W_expert[0,:]
  Gdot[b,i] = gpu_nodes[b,i,:]  . W_gpu[0,:]
  A[b,i]  = sum_j affinity[b,i,j]
  Bs[b,i] = sum_j bandwidth[b,i,j]
  Ts[b,i] = sum_j traffic[b,i,j]
  Se[b] = sum_i Edot[b,i] ;  Sg[b] = sum_i Gdot[b,i]
  h[b,i] = relu( c_pre_e*Edot + c_pre_g*Gdot + c_k0_e*Se + c_k0_g*Sg
                 + k_a*A + k_b*Bs + k_t*Ts )
  out[b,i,g] = mask[b,i,g] ? 0 : exp(h[b,i]*W2[g]) / Z[b,i]
  Z[b,i] = sum_g (1-mask) * exp(h[b,i]*W2[g])

Sharding: data-parallel over batch B=16 across 8 cores (2 batches/core).
"""
import sys

sys.path.insert(0, '/opt/trn_rl_repo')

import numpy as np

import concourse.bacc as bacc
import concourse.mybir as mybir
from concourse.bass_isa import ReduceOp
from concourse.bass_utils import run_bass_kernel_spmd
from concourse.tile import TileContext

B, N, DE, DG = 16, 2048, 16, 8
NCORES = 8
BB = B // NCORES          # batches per core
P = 128                   # partitions
TILES = N // P            # 16 row-tiles per batch

f32 = mybir.dt.float32
u8 = mybir.dt.uint8
AX = mybir.AxisListType
OP = mybir.AluOpType
AF = mybir.ActivationFunctionType


def _build_nc(consts):
    """Trace the per-core Bass kernel. `consts` carries the scalar weight
    constants baked in as immediates."""
    c_pre_e = float(consts["c_pre_e"])
    c_pre_g = float(consts["c_pre_g"])
    c_k0_e = float(consts["c_k0_e"])
    c_k0_g = float(consts["c_k0_g"])
    k_a = float(consts["k_a"])
    k_b = float(consts["k_b"])
    k_t = float(consts["k_t"])

    nc = bacc.Bacc("TRN2", target_bir_lowering=False, debug=False,
                   num_devices=NCORES)

    aff = nc.dram_tensor("affinity", [BB, N, N], f32, kind="ExternalInput")
    bwd = nc.dram_tensor("bandwidth", [BB, N, N], f32, kind="ExternalInput")
    trf = nc.dram_tensor("traffic", [BB, N, N], f32, kind="ExternalInput")
    msk = nc.dram_tensor("mask", [BB, N, N], u8, kind="ExternalInput")
    xe = nc.dram_tensor("xe", [BB, P, TILES, DE], f32, kind="ExternalInput")
    xg = nc.dram_tensor("xg", [BB, P, TILES, DG], f32, kind="ExternalInput")
    w2b = nc.dram_tensor("w2b", [P, N], f32, kind="ExternalInput")
    ueb = nc.dram_tensor("ueb", [P, TILES, DE], f32, kind="ExternalInput")
    ugb = nc.dram_tensor("ugb", [P, TILES, DG], f32, kind="ExternalInput")
    out_d = nc.dram_tensor("out", [BB, N, N], f32, kind="ExternalOutput")

    with TileContext(nc) as tc:
        with tc.tile_pool(name="const", bufs=1) as cpool, \
             tc.tile_pool(name="stream", bufs=2) as spool, \
             tc.tile_pool(name="mpool", bufs=4) as mpool, \
             tc.tile_pool(name="work", bufs=3) as wpool, \
             tc.tile_pool(name="small", bufs=6) as smpool:

            w2b_sb = cpool.tile([P, N], f32, tag="w2b")
            nc.gpsimd.dma_start(w2b_sb[:], w2b[:])
            ue_sb = cpool.tile([P, TILES, DE], f32, tag="ueb")
            nc.gpsimd.dma_start(ue_sb[:], ueb[:])
            ug_sb = cpool.tile([P, TILES, DG], f32, tag="ugb")
            nc.gpsimd.dma_start(ug_sb[:], ugb[:])

            # ---- stage 1: per-batch row scalars (pre[b] : [P, TILES]) ----
            pre = []
            for b in range(BB):
                xe_sb = cpool.tile([P, TILES, DE], f32, tag=f"xe{b}")
                nc.gpsimd.dma_start(xe_sb[:], xe[b])
                xg_sb = cpool.tile([P, TILES, DG], f32, tag=f"xg{b}")
                nc.gpsimd.dma_start(xg_sb[:], xg[b])

                prod_e = smpool.tile([P, TILES, DE], f32, tag="prod_e")
                nc.vector.tensor_mul(out=prod_e[:], in0=xe_sb[:], in1=ue_sb[:])
                edot = cpool.tile([P, TILES], f32, tag=f"edot{b}")
                nc.vector.tensor_reduce(out=edot[:], in_=prod_e[:],
                                        axis=AX.X, op=OP.add)
                prod_g = smpool.tile([P, TILES, DG], f32, tag="prod_g")
                nc.vector.tensor_mul(out=prod_g[:], in0=xg_sb[:], in1=ug_sb[:])
                gdot = cpool.tile([P, TILES], f32, tag=f"gdot{b}")
                nc.vector.tensor_reduce(out=gdot[:], in_=prod_g[:],
                                        axis=AX.X, op=OP.add)

                sep = smpool.tile([P, 1], f32, tag="sep")
                nc.vector.tensor_reduce(out=sep[:], in_=edot[:],
                                        axis=AX.X, op=OP.add)
                sgp = smpool.tile([P, 1], f32, tag="sgp")
                nc.vector.tensor_reduce(out=sgp[:], in_=gdot[:],
                                        axis=AX.X, op=OP.add)
                sea = smpool.tile([P, 1], f32, tag="sea")
                nc.gpsimd.partition_all_reduce(sea[:], sep[:], channels=P,
                                               reduce_op=ReduceOp.add)
                sga = smpool.tile([P, 1], f32, tag="sga")
                nc.gpsimd.partition_all_reduce(sga[:], sgp[:], channels=P,
                                               reduce_op=ReduceOp.add)

                k0 = smpool.tile([P, 1], f32, tag="k0")
                nc.vector.tensor_scalar(out=k0[:], in0=sea[:],
                                        scalar1=c_k0_e, scalar2=None,
                                        op0=OP.mult)
                k0b = cpool.tile([P, 1], f32, tag=f"k0b{b}")
                nc.vector.tensor_scalar(out=k0b[:], in0=sga[:],
                                        scalar1=c_k0_g, scalar2=k0[:, 0:1],
                                        op0=OP.mult, op1=OP.add)
                pre_b = cpool.tile([P, TILES], f32, tag=f"pre{b}")
                nc.vector.tensor_scalar(out=pre_b[:], in0=edot[:],
                                        scalar1=c_pre_e, scalar2=k0b[:, 0:1],
                                        op0=OP.mult, op1=OP.add)
                nc.vector.scalar_tensor_tensor(out=pre_b[:], in0=gdot[:],
                                               scalar=c_pre_g, in1=pre_b[:],
                                               op0=OP.mult, op1=OP.add)
                pre.append(pre_b)

            # ---- stage 2: stream the big tensors in double-height
            # tiles ([128, 2, 2048] = 2 MB per dma_start). Two-stage
            # software pipeline: loads + row-sum reduces (which free the
            # streaming tiles) are emitted one double-tile AHEAD of the
            # latency-heavy h->exp->mask->normalize->store chain, so the
            # per-engine queues prioritize slot-freeing work and DMA
            # never waits on the long chain. ----
            DT = TILES // 2                 # 8 double-tiles per batch

            def emit_loads_reds(b, dt):
                r0 = dt * 2 * P
                rows = slice(r0, r0 + 2 * P)
                a_t = spool.tile([P, 2, N], f32, tag="aff")
                nc.sync.dma_start(
                    a_t[:], aff[b, rows, :].rearrange("(u p) n -> p u n", p=P))
                b_t = spool.tile([P, 2, N], f32, tag="bw")
                nc.sync.dma_start(
                    b_t[:], bwd[b, rows, :].rearrange("(u p) n -> p u n", p=P))
                r_t = spool.tile([P, 2, N], f32, tag="tr")
                nc.sync.dma_start(
                    r_t[:], trf[b, rows, :].rearrange("(u p) n -> p u n", p=P))
                m_t = mpool.tile([P, 2, N], u8, tag="mask")
                nc.sync.dma_start(
                    m_t[:], msk[b, rows, :].rearrange("(u p) n -> p u n", p=P))

                Bs = smpool.tile([P, 2], f32, tag="Bs")
                nc.vector.tensor_reduce(out=Bs[:], in_=b_t[:],
                                        axis=AX.X, op=OP.add)
                Ts = smpool.tile([P, 2], f32, tag="Ts")
                nc.vector.tensor_reduce(out=Ts[:], in_=r_t[:],
                                        axis=AX.X, op=OP.add)
                As = []
                for j in range(2):
                    A = smpool.tile([P, 1], f32, tag=f"A{j}")
                    nc.scalar.activation(out=a_t[:, j, :], in_=a_t[:, j, :],
                                         func=AF.Copy, bias=0.0, scale=1.0,
                                         accum_out=A[:])
                    As.append(A)
                return dict(b=b, dt=dt, m_t=m_t, As=As, Bs=Bs, Ts=Ts)

            def emit_chain(st):
                b, dt, m_t = st["b"], st["dt"], st["m_t"]
                for j in range(2):
                    t = 2 * dt + j
                    rows_j = slice(t * P, (t + 1) * P)
                    h1 = smpool.tile([P, 1], f32, tag=f"h1{j}")
                    nc.vector.tensor_scalar(out=h1[:], in0=st["As"][j][:],
                                            scalar1=k_a,
                                            scalar2=pre[b][:, t:t + 1],
                                            op0=OP.mult, op1=OP.add)
                    h2 = smpool.tile([P, 1], f32, tag=f"h2{j}")
                    nc.vector.tensor_scalar(out=h2[:],
                                            in0=st["Bs"][:, j:j + 1],
                                            scalar1=k_b, scalar2=h1[:, 0:1],
                                            op0=OP.mult, op1=OP.add)
                    h3 = smpool.tile([P, 1], f32, tag=f"h3{j}")
                    nc.vector.tensor_scalar(out=h3[:],
                                            in0=st["Ts"][:, j:j + 1],
                                            scalar1=k_t, scalar2=h2[:, 0:1],
                                            op0=OP.mult, op1=OP.add)
                    hr = smpool.tile([P, 1], f32, tag=f"hr{j}")
                    nc.vector.tensor_scalar_max(out=hr[:], in0=h3[:],
                                                scalar1=0.0)

                    Eh = wpool.tile([P, N], f32, tag=f"E{j}")
                    nc.scalar.activation(out=Eh[:], in_=w2b_sb[:],
                                         func=AF.Exp, bias=0.0,
                                         scale=hr[:, 0:1])
                    Z = smpool.tile([P, 1], f32, tag=f"Z{j}")
                    nc.vector.scalar_tensor_tensor(
                        out=Eh[:], in0=m_t[:, j, :], scalar=1.0,
                        in1=Eh[:], op0=OP.not_equal, op1=OP.mult,
                        accum_out=Z[:])
                    R = smpool.tile([P, 1], f32, tag=f"R{j}")
                    nc.vector.reciprocal(R[:], Z[:])
                    nc.scalar.activation(out=Eh[:], in_=Eh[:],
                                         func=AF.Copy, bias=0.0,
                                         scale=R[:, 0:1])
                    nc.scalar.dma_start(out_d[b, rows_j, :], Eh[:])

            for b in range(BB):
                for dt in range(DT):
                    emit_chain(emit_loads_reds(b, dt))

    nc.compile()
    return nc


def _ensure_ntff_hook():
    """The agent image's antenv lacks axon_hooks; inject it and register the
    boot script's ctypes NTFF hook so trace=True works."""
    import types
    if "antenv.axon_hooks" in sys.modules:
        return
    mod = types.ModuleType("antenv.axon_hooks")
    mod._hook = None

    def set_axon_ntff_profile_hook(h):
        mod._hook = h

    def get_axon_ntff_profile_hook():
        return mod._hook

    mod.set_axon_ntff_profile_hook = set_axon_ntff_profile_hook
    mod.get_axon_ntff_profile_hook = get_axon_ntff_profile_hook
    sys.modules["antenv.axon_hooks"] = mod
    try:
        from trn_agent_boot.trn_boot import _ntff_profile_via_ctypes
        mod._hook = _ntff_profile_via_ctypes('/opt/axon/libaxon_pjrt.so')
    except Exception:
        pass


def run(inputs, trace=False):
    """Shard inputs over 8 cores, run the Bass kernel, gather the output.
    Returns (full_output, BassKernelResults)."""
    if trace:
        _ensure_ntff_hook()
    xe = np.asarray(inputs["expert_node"], np.float32)
    xg = np.asarray(inputs["gpu_nodes"], np.float32)
    aff = np.asarray(inputs["affinity"], np.float32)
    bwd = np.asarray(inputs["bandwidth"], np.float32)
    trf = np.asarray(inputs["traffic"], np.float32)
    msk = np.asarray(inputs["mask_gpu_action"]).astype(np.uint8)
    W_expert = np.asarray(inputs["W_expert"], np.float32)
    W_gpu = np.asarray(inputs["W_gpu"], np.float32)
    w_eatt = np.asarray(inputs["w_eatt"], np.float32)
    w_gatt = np.asarray(inputs["w_gatt"], np.float32)
    W_actor1 = np.asarray(inputs["W_actor1"], np.float32)
    W_actor2 = np.asarray(inputs["W_actor2"], np.float32)

    wa, wb, wc = w_eatt[0, 0], w_eatt[0, 1], w_eatt[0, 2]
    ga, gb = w_gatt[0, 0], w_gatt[0, 1]
    gbw, gtr = w_gatt[0, 2], w_gatt[0, 3]
    w10, w11 = W_actor1[0, 0], W_actor1[0, 1]

    consts = {
        "c_pre_e": w10 * N * wa,
        "c_pre_g": w11 * N * ga,
        "c_k0_e": w10 * wb,
        "c_k0_g": w11 * gb,
        "k_a": w10 * wc,
        "k_b": w11 * gbw,
        "k_t": w11 * gtr,
    }

    u_e = W_expert[0]                          # [DE]
    u_g = W_gpu[0]                             # [DG]
    W2 = W_actor2[:, 0]                        # [N]
    w2b = np.ascontiguousarray(np.repeat(W2[None, :], P, 0))
    ueb = np.ascontiguousarray(
        np.broadcast_to(u_e[None, None, :], (P, TILES, DE)))
    ugb = np.ascontiguousarray(
        np.broadcast_to(u_g[None, None, :], (P, TILES, DG)))
    # [BB,N,D] -> [BB,P,TILES,D] so partition p / column t holds row t*128+p
    xe_r = np.ascontiguousarray(
        xe.reshape(B, TILES, P, DE).transpose(0, 2, 1, 3))
    xg_r = np.ascontiguousarray(
        xg.reshape(B, TILES, P, DG).transpose(0, 2, 1, 3))

    nc = _build_nc(consts)

    in_maps = []
    for c in range(NCORES):
        s = slice(c * BB, (c + 1) * BB)
        in_maps.append({
            "affinity": aff[s], "bandwidth": bwd[s], "traffic": trf[s],
            "mask": msk[s], "xe": xe_r[s], "xg": xg_r[s],
            "w2b": w2b, "ueb": ueb, "ugb": ugb,
        })

    res = run_bass_kernel_spmd(nc, in_maps, list(range(NCORES)), trace=trace)
    out = np.concatenate([res.results[c]["out"] for c in range(NCORES)],
                         axis=0)
    return out, res


def kernel(**inputs):
    out, _ = run(inputs, trace=False)
    return out
